# revision 16
# baseline (speedup 1.0000x reference)
"""Single-head causal attention (B=8, S=2048, D=1024) on 8 Trainium2 cores.

Strategy: pure data-parallel over batch — core b computes attention for
batch element b end-to-end (no collectives). All matmuls run in FP32r
(fp32 with 11 explicit mantissa bits, full PE rate at moving-dim >= 256;
~16x more accurate than bf16 at the same speed). Measured relative error
vs the fp32 reference: ~2.5e-4. Modeled per-core time ~340 us; measured
steady-state per-iteration ~0.34 ms (ambient-load dependent).

Host-side prep (part of kernel()): shard batch across cores, transpose
X to X^T, round X^T/W to f32r (nearest-even), pre-scale bq by 1/sqrt(D),
build the causal additive mask tiles. bk is dropped entirely — softmax is
invariant to the per-row constant q.bk.

Per-core device pipeline:
  Phase A (projections, software-pipelined over 256-column groups):
    K^T [e, sk] (bias-free, e-tile pairs batched per PSUM bank) and
    V [sk, dv] stay resident in SBUF; Q^T (+bq/sqrt(D), x 1/sqrt(D))
    bounces through a DRAM buffer. PSUM rotates over 6 banks; copies
    alternate ACT/DVE.
  Phase B (per 128-row query tile, software-pipelined, exp deferred to
    after the previous tile's PV so PT copies aren't queued behind it):
    scores = Q^T.T @ K^T causal chunks (128-wide diagonal chunks widened
    to 256 to stay at full f32r rate) -> additive tril mask -> rowmax
    (DVE) / exp+rowsum (ACT) -> PE-transpose P -> P^T @ V accumulation ->
    scale by 1/rowsum (DVE), add bv (GPSIMD), store.
"""

import os
import sys

sys.path.insert(0, "/opt/trn_rl_repo")

import numpy as np

import concourse.bacc as bacc
import concourse.tile as tile
from concourse import mybir
from concourse.bass import ds, ts
import concourse.bass as bass
from concourse.bass_utils import run_bass_kernel_spmd

F32 = mybir.dt.float32
F32R = mybir.dt.float32r

B, S, D = 8, 2048, 1024
P = 128                     # partition width
DT = D // P                 # 8 d-tiles (contraction)
ET = D // P                 # 8 e-tiles (output feature tiles)
ST = S // P                 # 16 s-tiles
GROUP_S = 256               # s-rows per phase-A group
NG = S // GROUP_S           # 8 groups
NEG = -1.0e30

USE_F32R = os.environ.get("ATTN_NO_F32R", "") == ""
MM_DT = F32R if USE_F32R else F32


def _phase_a(nc, tc, ext, consts, kt_sb, v_sb, qt_dram, ps_tr, ps_mm, ps_pv,
             qt_fetch):
    """Projections from host-transposed X^T: fill kt_sb, v_sb, qt_dram.

    bk is never applied: softmax is invariant to the per-row constant q.bk.
    X^T and W arrive pre-rounded (f32r) from the host, so matmul operands
    come straight from DMA.
    """
    bias_sb = consts[2]
    with (
        tc.tile_pool(name="pha_w", bufs=1) as pha_w,
        tc.tile_pool(name="pha_s", bufs=3) as pha_s,
        tc.tile_pool(name="pha_q", bufs=3) as pha_q,
    ):
        wr = pha_w.tile([P, DT, D], MM_DT, tag="wr")

        def load_w(w_ext):
            for d in range(DT):
                nc.sync.dma_start(out=wr[:, d, :], in_=w_ext[ts(d, P), :])

        def stage_load(xt_ext, g):
            """DMA X^T columns [g*256, (g+1)*256) -> XT [p, d-tile, 256]."""
            xt_t = pha_s.tile([P, DT, GROUP_S], MM_DT, tag="xt")
            nc.sync.dma_start(
                out=xt_t,
                in_=xt_ext[:, ds(g * GROUP_S, GROUP_S)].rearrange(
                    "(dt p) s -> p dt s", p=P
                ),
            )
            return xt_t

        def stage_mm(proj, g, xt_t):
            if proj == "v":
                for ss in range(GROUP_S // P):
                    t_idx = g * (GROUP_S // P) + ss
                    for dv in range(2):
                        pool = ps_mm if dv == 0 else ps_tr
                        vp = pool.tile([P, 512], F32, tag="mm" if dv == 0 else "tr")
                        for d in range(DT):
                            nc.tensor.matmul(
                                vp,
                                xt_t[:, d, ts(ss, P)],
                                wr[:, d, ts(dv, 512)],
                                start=(d == 0),
                                stop=(d == DT - 1),
                            )
                        if dv == 0:
                            nc.scalar.copy(out=v_sb[:, t_idx, ts(dv, 512)], in_=vp)
                        else:
                            nc.vector.tensor_copy(
                                out=v_sb[:, t_idx, ts(dv, 512)], in_=vp
                            )
            elif proj == "k":
                for ep in range(ET // 2):  # pairs of e-tiles share a PSUM bank
                    pool, tag = [(ps_mm, "mm"), (ps_tr, "tr"), (ps_pv, "pv")][ep % 3]
                    pp = pool.tile([P, 2, GROUP_S], F32, tag=tag)
                    for h in range(2):
                        e = ep * 2 + h
                        for d in range(DT):
                            nc.tensor.matmul(
                                pp[:, h, :],
                                wr[:, d, ts(e, P)],
                                xt_t[:, d, :],
                                start=(d == 0),
                                stop=(d == DT - 1),
                            )
                    if ep % 2 == 0:
                        nc.scalar.copy(
                            out=kt_sb[:, ds(ep * 2, 2), ds(g * GROUP_S, GROUP_S)],
                            in_=pp,
                        )
                    else:
                        nc.vector.tensor_copy(
                            out=kt_sb[:, ds(ep * 2, 2), ds(g * GROUP_S, GROUP_S)],
                            in_=pp,
                        )
            else:  # q: bias bq/sqrt(D) (host-scaled); 1/sqrt(D) scale on scores
                for ep in range(ET // 2):
                    qt_stage = pha_q.tile([P, 2, GROUP_S], MM_DT, tag="qstage")
                    for h in range(2):
                        e = ep * 2 + h
                        pool = ps_mm if h == 0 else ps_tr
                        pp = pool.tile([P, GROUP_S], F32, tag="mm" if h == 0 else "tr")
                        for d in range(DT):
                            nc.tensor.matmul(
                                pp,
                                wr[:, d, ts(e, P)],
                                xt_t[:, d, :],
                                start=(d == 0),
                                stop=(d == DT - 1),
                            )
                        if h == 0:
                            nc.scalar.activation(
                                out=qt_stage[:, h, :],
                                in_=pp,
                                func=mybir.ActivationFunctionType.Identity,
                                bias=bias_sb[:, 1, ds(e, 1)],
                                scale=float(1.0 / np.sqrt(D)),
                            )
                        else:
                            nc.vector.tensor_scalar(
                                out=qt_stage[:, h, :],
                                in0=pp,
                                scalar1=float(1.0 / np.sqrt(D)),
                                scalar2=bias_sb[:, 1, ds(e, 1)],
                                op0=mybir.AluOpType.mult,
                                op1=mybir.AluOpType.add,
                            )
                    nc.sync.dma_start(
                        out=qt_dram[:, ds(ep * 2, 2), ds(g * GROUP_S, GROUP_S)],
                        in_=qt_stage,
                    )

        # Q before V so the Q^T DRAM bounce completes well before phase B.
        for proj in ("k", "q", "v"):
            prev = None
            for g in range(NG):
                xt_t = stage_load(ext["x" + proj + "t"], g)
                if g == 0:
                    load_w(ext["w" + proj])
                if prev is not None:
                    stage_mm(proj, *prev)
                prev = (g, xt_t)
            stage_mm(proj, *prev)
            if proj == "q":
                qt_fetch(0)  # prefetch first phase-B Q^T tile during V pass


def _phase_b(nc, tc, out_ext, consts, kt_sb, v_sb, qt_dram, ps_tr, ps_mm, ps_pv,
             qt_tiles, qt_fetch):
    ident_sb, maskc_sb, bias_sb, bv_sb, maskw_sb = consts
    with (
        tc.tile_pool(name="phb1", bufs=1) as phb1,
        tc.tile_pool(name="phb", bufs=2) as phb,
    ):

        def softmax_part(i):
            """scores + softmax for q-tile i; returns (p_sb, stats)."""
            L = (i + 1) * P
            n_chunks = (L + 511) // 512
            qt_t = qt_tiles.pop(i)
            if i + 1 < ST:
                qt_fetch(i + 1)

            sc_sb = phb.tile([P, S], F32, tag="scores")
            L_eff = L
            for c in range(n_chunks):
                cs = c * 512
                w = min(512, L - cs)
                last = c == n_chunks - 1
                mask_ap = None
                if last:
                    if w == 128:
                        # widen to 256: f32r matmul is 4 cyc/row below 256
                        w = 256
                        L_eff = cs + 256
                        mask_ap = maskw_sb[:, :]
                    else:
                        mask_ap = maskc_sb[:, ds(512 - w, w)]
                sp = ps_mm.tile([P, 512], F32, tag="mm")
                for e in range(ET):
                    nc.tensor.matmul(
                        sp[:, :w],
                        qt_t[:, e, :],
                        kt_sb[:, e, ds(cs, w)],
                        start=(e == 0),
                        stop=(e == ET - 1),
                    )
                if last:
                    nc.vector.tensor_add(
                        out=sc_sb[:, ds(cs, w)], in0=sp[:, :w], in1=mask_ap
                    )
                else:
                    nc.vector.tensor_copy(out=sc_sb[:, ds(cs, w)], in_=sp[:, :w])

            stats = phb.tile([P, 4], F32, tag="stats")
            nc.vector.reduce_max(
                out=stats[:, 0:1],
                in_=sc_sb[:, :L_eff],
                axis=mybir.AxisListType.X,
                negate=True,
            )
            return sc_sb, stats, L_eff

        def exp_part(i, sc_sb, stats, L_eff):
            p_sb = phb.tile([P, S], MM_DT, tag="p")
            nc.scalar.activation(
                out=p_sb[:, :L_eff],
                in_=sc_sb[:, :L_eff],
                func=mybir.ActivationFunctionType.Exp,
                bias=stats[:, 0:1],
                scale=1.0,
                accum_out=stats[:, 1:2],
            )
            nc.vector.reciprocal(out=stats[:, 2:3], in_=stats[:, 1:2])
            return p_sb, stats

        def pv_part(i, p_sb, stats):
            """P^T, P^T @ V, normalize, +bv, store for q-tile i."""
            n_k = i + 1
            pt_t = phb1.tile([P, ST, P], MM_DT, tag="pt")
            for tb in range((n_k + 3) // 4):
                nb = min(4, n_k - tb * 4)
                trp = ps_tr.tile([P, 512], MM_DT, tag="tr")
                for k4 in range(nb):
                    nc.tensor.transpose(
                        out=trp[:, ts(k4, P)],
                        in_=p_sb[:, ts(tb * 4 + k4, P)],
                        identity=ident_sb,
                    )
                if tb % 2 == 0:
                    nc.scalar.copy(
                        out=pt_t[:, ds(tb * 4, nb), :],
                        in_=trp[:, ds(0, nb * P)]
                        .bitcast(F32)
                        .rearrange("p (a b) -> p a b", a=nb),
                    )
                else:
                    nc.vector.tensor_copy(
                        out=pt_t[:, ds(tb * 4, nb), :],
                        in_=trp[:, ds(0, nb * P)]
                        .bitcast(F32)
                        .rearrange("p (a b) -> p a b", a=nb),
                    )

            out_sb = phb.tile([P, D], F32, tag="osb")
            for dv in range(2):
                pvp = ps_pv.tile([P, 512], F32, tag="pv")
                for t in range(n_k):
                    nc.tensor.matmul(
                        pvp,
                        pt_t[:, t, :],
                        v_sb[:, t, ts(dv, 512)],
                        start=(t == 0),
                        stop=(t == n_k - 1),
                    )
                nc.vector.tensor_scalar_mul(
                    out=out_sb[:, ts(dv, 512)], in0=pvp, scalar1=stats[:, 2:3]
                )
                nc.gpsimd.tensor_add(
                    out=out_sb[:, ts(dv, 512)],
                    in0=out_sb[:, ts(dv, 512)],
                    in1=bv_sb[:, ts(dv, 512)],
                )
            nc.sync.dma_start(out=out_ext[ts(i, P), :], in_=out_sb)

        prev = None
        for i in range(ST):
            sc_sb, stats, L_eff = softmax_part(i)
            if prev is not None:
                pv_part(*prev)
            prev = (i, *exp_part(i, sc_sb, stats, L_eff))
        pv_part(*prev)


def _build(nc, repeat=1):
    ext = {}
    ext["xqt"] = nc.declare_dram_parameter("xqt", [D, S], MM_DT, isOutput=False)
    ext["xkt"] = nc.declare_dram_parameter("xkt", [D, S], MM_DT, isOutput=False)
    ext["xvt"] = nc.declare_dram_parameter("xvt", [D, S], MM_DT, isOutput=False)
    ext["wq"] = nc.declare_dram_parameter("wq", [D, D], MM_DT, isOutput=False)
    ext["wk"] = nc.declare_dram_parameter("wk", [D, D], MM_DT, isOutput=False)
    ext["wv"] = nc.declare_dram_parameter("wv", [D, D], MM_DT, isOutput=False)
    bq = nc.declare_dram_parameter("bq", [D], F32, isOutput=False)  # pre-scaled
    bv = nc.declare_dram_parameter("bv", [D], F32, isOutput=False)
    # [128, 512] additive mask; cols 384..511 hold the tril block, rest 0
    maskc = nc.declare_dram_parameter("maskc", [P, 512], F32, isOutput=False)
    # [128, 256] additive mask for widened 128-col diagonal chunks
    maskw = nc.declare_dram_parameter("maskw", [P, 256], F32, isOutput=False)
    ident = nc.declare_dram_parameter("ident", [P, P], MM_DT, isOutput=False)
    out_ext = nc.declare_dram_parameter("out", [S, D], F32, isOutput=True)

    qt_dram = nc.dram_tensor("qt_bounce", [P, ET, S], MM_DT)

    with tile.TileContext(nc) as tc:
        with (
            tc.tile_pool(name="res", bufs=1) as res,          # long-lived
            tc.tile_pool(name="ps_tr", bufs=2, space="PSUM") as ps_tr,
            tc.tile_pool(name="ps_mm", bufs=4, space="PSUM") as ps_mm,
            tc.tile_pool(name="ps_pv", bufs=2, space="PSUM") as ps_pv,
        ):
            kt_sb = res.tile([P, ET, S], MM_DT, tag="kt")     # K^T [e, sk]
            v_sb = res.tile([P, ST, D], MM_DT, tag="v")       # V [sk, dv]

            ident_sb = res.tile([P, P], MM_DT, tag="ident")
            nc.sync.dma_start(out=ident_sb, in_=ident[:, :])

            bias_sb = res.tile([P, 3, ET], F32, tag="bias")   # [:,1,:] = bq/sqrt(D)
            for e in range(ET):
                nc.gpsimd.dma_start(out=bias_sb[:, 1, ds(e, 1)], in_=bq[ts(e, P)])

            maskc_sb = res.tile([P, 512], F32, tag="maskc")
            nc.gpsimd.dma_start(out=maskc_sb, in_=maskc[:, :])
            maskw_sb = res.tile([P, 256], F32, tag="maskw")
            nc.gpsimd.dma_start(out=maskw_sb, in_=maskw[:, :])
            bv_sb = res.tile([P, D], F32, tag="bv")
            bv_ap = bv[:]
            bv_bcast = bass.AP(
                tensor=bv_ap.tensor, offset=bv_ap.offset, ap=[[0, P], [1, D]]
            )
            nc.gpsimd.dma_start(out=bv_sb, in_=bv_bcast)

            consts = (ident_sb, maskc_sb, bias_sb, bv_sb, maskw_sb)
            with tc.tile_pool(name="qtp", bufs=2) as qt_pool:
                qt_tiles = {}

                def qt_fetch(i):
                    t = qt_pool.tile([P, ET, P], MM_DT, tag="qt")
                    nc.sync.dma_start(out=t, in_=qt_dram[:, :, ts(i, P)])
                    qt_tiles[i] = t

                for _rep in range(repeat):
                    _phase_a(
                        nc, tc, ext, consts, kt_sb, v_sb, qt_dram,
                        ps_tr, ps_mm, ps_pv, qt_fetch,
                    )
                    _phase_b(
                        nc, tc, out_ext, consts, kt_sb, v_sb, qt_dram,
                        ps_tr, ps_mm, ps_pv, qt_tiles, qt_fetch,
                    )

    nc.compile()
    return nc


_NC_CACHE = {}


def _get_nc(repeat=1):
    if repeat not in _NC_CACHE:
        nc = bacc.Bacc("TRN2", target_bir_lowering=False)
        _NC_CACHE[repeat] = _build(nc, repeat=repeat)
    return _NC_CACHE[repeat]


def _round_f32r(a):
    """Round fp32 to f32r (11 explicit mantissa bits), nearest-even."""
    if not USE_F32R:
        return np.ascontiguousarray(a, np.float32)
    u = np.ascontiguousarray(a, np.float32).view(np.uint32)
    r = u + np.uint32(0x7FF) + ((u >> np.uint32(12)) & np.uint32(1))
    r &= np.uint32(0xFFFFF000)
    return r.view(np.float32)


def _host_inputs(query, key, value, mask, Wq, bq, Wk, bk, Wv, bv):
    tril = np.tril(np.ones((S, S), dtype=bool))
    if not np.array_equal(np.asarray(mask, dtype=bool), tril):
        raise ValueError("kernel is specialized to the causal (tril) mask")

    row = np.arange(P)[:, None]
    col = np.arange(P)[None, :]
    tril_add = np.where(row >= col, 0.0, NEG).astype(np.float32)
    maskc = np.concatenate(
        [np.zeros((P, 512 - P), np.float32), tril_add], axis=1
    )
    maskw = np.concatenate(
        [tril_add, np.full((P, 128), NEG, np.float32)], axis=1
    )
    ident = _round_f32r(np.eye(P, dtype=np.float32))

    shared = {
        "wq": _round_f32r(Wq),
        "wk": _round_f32r(Wk),
        "wv": _round_f32r(Wv),
        # bq pre-scaled by 1/sqrt(D); bk dropped (softmax row-shift invariance)
        "bq": (np.asarray(bq, np.float32) / np.float32(np.sqrt(D))).astype(
            np.float32
        ),
        "bv": np.ascontiguousarray(bv, np.float32),
        "maskc": maskc,
        "maskw": maskw,
        "ident": ident,
    }
    q_all = np.asarray(query, np.float32)
    k_all = np.asarray(key, np.float32)
    v_all = np.asarray(value, np.float32)
    in_maps = []
    for b in range(B):
        m = dict(shared)
        m["xqt"] = _round_f32r(q_all[b].T)
        m["xkt"] = _round_f32r(k_all[b].T)
        m["xvt"] = _round_f32r(v_all[b].T)
        in_maps.append(m)
    return in_maps


def run(inputs, trace=False, repeat=1, **spmd_kwargs):
    nc = _get_nc(repeat)
    in_maps = _host_inputs(**inputs)
    res = run_bass_kernel_spmd(
        nc, in_maps, list(range(B)), trace=trace, **spmd_kwargs
    )
    out = np.stack([res.results[c]["out"] for c in range(B)], axis=0)
    return out.astype(np.float32), res


def kernel(**inputs) -> np.ndarray:
    out, _ = run(inputs, trace=False)
    return out
